# revision 26
# baseline (speedup 1.0000x reference)
"""CWCT (class-wise whitening/coloring transform) for Trainium2, 8 NeuronCores.

Strategy
--------
Pixels are counting-sorted by segment label on the host (pure data
movement); each label's pixel range is split contiguously across the 8
cores, zero-padded to a fixed per-(core,label) capacity C.

Device phase 1 (per core): for every label, accumulate the raw second
moment S_l = sum_p x_p x_p^T and the channel sums over that core's pixel
shard, for content and style, as grouped 128-pixel-contraction matmuls
into PSUM (fp8e3 operands — 4 mantissa bits, f32 accumulate). A
ones-column appended to the gathered arrays yields the channel sums for
free in the same matmuls. Content streams on the SP(sync) DMA ring,
style on the ACT(scalar) ring so both HWDGE queues pull concurrently.
Partial moments are evicted as f16 (safe: partial sums are ~4e3 max and
the host all-reduce runs in f64).

Host middle: all-reduce the (tiny) per-core partial moments, form
covariances, Cholesky factors, inv_Lc via triangular solve (float64).
The transform is shipped in DELTA form: T' = 8*(T - I) quantized e4m3
and b' = 8*b, where T = Ls @ inv_Lc.

Device phase 2 (per core): delta = T' @ x + b' with x in e4m3 and
T' the DoubleRow-fp8 stationary (contraction 256 = 128 partitions x 2
k-tiles per single matmul), delta evicted as fp8e3. 1 byte/element on
both the input and output streams.

Host end: out = content_f32 + delta/8 scattered back to original pixel
order. Reconstructing against the exact f32 content means the x
quantization error only survives through (T - I) (spectral norm ~0.12),
so e4m3 x costs ~0.1% end-to-end error.
"""

import numpy as np
import ml_dtypes

import concourse.bacc as bacc
import concourse.mybir as mybir
import concourse.tile as tile
from concourse.bass_utils import run_bass_kernel_spmd

NCORES = 8
E3 = ml_dtypes.float8_e3m4
E4 = ml_dtypes.float8_e4m3
F16 = np.float16

# set by test harness to capture profiles
TRACE = False
TRACE_DIR = "/tmp/cwct_trace"
LAST_NS = {}
# overlap phase-2's NEFF compile (background thread + dummy run) with phase 1
PRECOMPILE_WARM = True
DELTA_SCALE = 8.0


def _round_up(x, m):
    return (int(x) + m - 1) // m * m


P1_GK = 16  # phase-1 DMA group size in 128-px tiles (526KB per DMA, 4KB rows)


def _pin_xpin(nc, L, C):
    """Reserve the cross-NEFF persistent SBUF region holding the phase-2
    moving operand x: [128(k), L, 2(j), C] e4m3 = channel j*128+k of pixel
    l*C+px. The label/pair axes keep every matmul AP stride <= C (the
    matmul ISA static-pattern step field is 16-bit). Fixed top-of-SBUF
    offset, identical in both phases (same formula, same L*C). Phase 1
    writes it with its spare DMA bandwidth; phase 2 only reads it. SBUF
    contents persist across NEFF executions on this stack (probed); tile
    pools bump-allocate from the bottom and stay far below."""
    size = 2 * L * C  # bytes per partition (e4m3)
    off = (229344 - size) // 4096 * 4096
    return nc.alloc_sbuf_tensor_at(
        "xpin", [128, L, 2, C], mybir.dt.float8e4, offset=off
    )


def _p1_groups(T1):
    """Phase-1 DMA group tile counts per (feature, label): uniform small
    groups so the first matmul starts ~1.3us after the first DMA and the
    pipeline stays fed."""
    kts = []
    rem = T1
    while rem > 0:
        kts.append(min(P1_GK, rem))
        rem -= P1_GK
    return kts


def _build_phase1(L, C, N, T1c, T1s):
    """Inputs gc/gs: (L, LBLK) fp8e3, host-swizzled pixel-major gathered
    tiles (+ones column); per label, _p1_groups(T1x[l]) DMA groups each
    laid out (128, KT, N+1) so one DMA pulls KT*(N+1) contiguous bytes per
    SBUF partition. T1c/T1s give the per-label live tile count (trailing
    all-zero pad tiles are neither DMA'd nor matmul'd).
    Outputs sc/ss: (L, 128, 386) f16 per label row block:
    [:, 0:256]   = S[0:128, 0:256] (upper row block, all columns)
    [:, 256]     = channel sums for channels 0..127
    [:, 257:385] = S[128:256, 128:256] (lower-right block)
    [:, 385]     = channel sums for channels 128..255
    (S[128:256, 0:128] is recovered on the host as S[0:128,128:256].T)"""
    assert N == 256
    T1 = C // 128
    W = 2 * (N + 1) - 128  # 386
    LBLK = T1 * 128 * (N + 1)
    P2 = L * C
    nc = bacc.Bacc("TRN2", target_bir_lowering=False, debug=False, num_devices=NCORES)
    gc = nc.dram_tensor("gc", [L, LBLK], mybir.dt.float8e3, kind="ExternalInput")
    gs = nc.dram_tensor("gs", [L, LBLK], mybir.dt.float8e3, kind="ExternalInput")
    g2 = nc.dram_tensor("g2", [N, P2], mybir.dt.float8e4, kind="ExternalInput")
    sc = nc.dram_tensor("sc", [L, 128, W], mybir.dt.float16, kind="ExternalOutput")
    ss = nc.dram_tensor("ss", [L, 128, W], mybir.dt.float16, kind="ExternalOutput")

    with tile.TileContext(nc) as tc:
        xpin = _pin_xpin(nc, L, C)
        with (
            tc.tile_pool(name="gin", bufs=12) as gin,
            tc.tile_pool(name="out", bufs=4) as outp,
            tc.tile_pool(name="ps", bufs=8, space="PSUM") as psum,
        ):
            # content on the SP(sync) HWDGE ring, style on the ACT(scalar)
            # ring; per-label interleave keeps both rings streaming and the
            # PE alternating between the two moment chains.
            for l in range(L):
                for g_dram, o_dram, ineng, T1l in (
                    (gc, sc, nc.sync, T1c[l]),
                    (gs, ss, nc.scalar, T1s[l]),
                ):
                    ps0 = psum.tile([128, N + 1], mybir.dt.float32, tag="ps")
                    ps1 = psum.tile([128, 129], mybir.dt.float32, tag="ps")
                    n = 0
                    off = 0
                    for KT in _p1_groups(T1l):
                        t = gin.tile([128, P1_GK, N + 1], mybir.dt.float8e3, tag="g")
                        src = g_dram[l, off : off + 128 * KT * (N + 1)].rearrange(
                            "(p t c) -> p t c", p=128, t=KT, c=N + 1
                        )
                        ineng.dma_start(t[:, 0:KT, :], src)
                        off += 128 * KT * (N + 1)
                        for k in range(KT):
                            nc.tensor.matmul(
                                ps0[:], t[:, k, 0:128], t[:, k, :],
                                start=(n == 0), stop=(n == T1l - 1),
                            )
                            nc.tensor.matmul(
                                ps1[:], t[:, k, 128:256], t[:, k, 128 : N + 1],
                                start=(n == 0), stop=(n == T1l - 1),
                            )
                            n += 1
                    ob = outp.tile([128, W], mybir.dt.float16, tag="o")
                    nc.vector.tensor_copy(ob[:, 0 : N + 1], ps0[:])
                    nc.vector.tensor_copy(ob[:, N + 1 : W], ps1[:])
                    # moment outputs on the SWDGE queue: keeps both HWDGE
                    # rings pure input streams for the PE
                    nc.gpsimd.dma_start(o_dram[l], ob[:])
                # preload one label-sized chunk of phase-2's x into the
                # persistent SBUF region, also via SWDGE: uses the DMA
                # engine slack of the PE-bound moment loop without sitting
                # ahead of moment loads in the HWDGE FIFOs
                nc.gpsimd.dma_start(
                    xpin[:, l, 0, :], g2[0:128, l * C : (l + 1) * C]
                )
                nc.gpsimd.dma_start(
                    xpin[:, l, 1, :], g2[128:256, l * C : (l + 1) * C]
                )
    nc.compile()
    return nc


def _build_phase2(L, C, N):
    """tq: (128, L, 2, 2, 128) e4m3 with tq[k,l,j,i,m] = T'_l[i*128+m, j*128+k]
        where T' = DELTA_SCALE*(T_l - I); the (j=2) axis is the DoubleRow
        k-tile pair, so one matmul contracts all 256 channels.
    oc: (N, L*C) fp8e3 delta output (channel-major, gathered order).
    The moving operand x is READ FROM THE PERSISTENT SBUF REGION that
    phase 1 preloaded (no input DMA at all). The bias DELTA_SCALE*b is
    folded into the host reconstruction, so evictions are pure
    PSUM->fp8 copies."""
    assert N == 256
    P2 = L * C
    assert C % 128 == 0

    nc = bacc.Bacc("TRN2", target_bir_lowering=False, debug=False, num_devices=NCORES)
    tq = nc.dram_tensor("tq", [128, L, 2, 2, 128], mybir.dt.float8e4, kind="ExternalInput")
    oc = nc.dram_tensor("oc", [N, P2], mybir.dt.float8e3, kind="ExternalOutput")

    with tile.TileContext(nc) as tc:
        xpin = _pin_xpin(nc, L, C)
        with (
            tc.tile_pool(name="const", bufs=1) as constp,
            tc.tile_pool(name="out", bufs=4) as outp,
            tc.tile_pool(name="ps", bufs=8, space="PSUM") as psum,
        ):
            # stationary transforms on the sync ring, per-label slices so
            # the first matmul only waits for label 0's 64KB
            tqt = constp.tile([128, L, 2, 2, 128], mybir.dt.float8e4)
            for l in range(L):
                nc.sync.dma_start(tqt[:, l], tq[:, l])

            ocr2 = oc[:].rearrange("(i k) x -> k i x", i=2)
            # output groups of 4096 px (4KB DMA descriptor rows); PSUM-bank-
            # limited sub-blocks of <=512 px per matmul (one DoubleRow matmul
            # contracts all 256 channels). Evictions rotate 2:1 over DVE/ACT.
            GSZ = 4096
            groups = []
            off = 0
            while off < C:
                g = min(GSZ, C - off)
                subs = []
                so = 0
                while so < g:
                    s = min(512, g - so)
                    subs.append((so, s))
                    so += s
                groups.append((off, g, subs))
                off += g
            ev = 0
            for l in range(L):
                for off, G, subs in groups:
                    # both i-chunks evict into one tile -> a single output
                    # DMA per group
                    ob = outp.tile([128, 2, GSZ], mybir.dt.float8e3, tag="o")
                    for i in range(2):
                        for so, S in subs:
                            base = off + so
                            ps = psum.tile([128, 512], mybir.dt.float32, tag="ps")
                            nc.tensor.matmul(
                                ps[:, 0:S], tqt[:, l, :, i, :],
                                xpin[:, l, :, base : base + S], start=True, stop=True,
                                perf_mode=mybir.MatmulPerfMode.DoubleRow,
                            )
                            if ev % 2 == 0:
                                nc.vector.tensor_copy(
                                    ob[:, i, so : so + S], ps[:, 0:S]
                                )
                            else:
                                nc.scalar.activation(
                                    ob[:, i, so : so + S], ps[:, 0:S],
                                    mybir.ActivationFunctionType.Identity,
                                )
                            ev += 1
                    nc.sync.dma_start(
                        ocr2[:, :, l * C + off : l * C + off + G], ob[:, :, 0:G]
                    )
    nc.compile()
    return nc


def _run(nc, in_maps, label):
    if TRACE:
        import os
        import shutil

        tdir = f"{TRACE_DIR}/{label}"
        shutil.rmtree(tdir, ignore_errors=True)
        os.makedirs(tdir, exist_ok=True)
        res = run_bass_kernel_spmd(
            nc, in_maps, list(range(NCORES)), trace=True, tmpdir=tdir
        )
        LAST_NS[label] = res.exec_time_ns
    else:
        res = run_bass_kernel_spmd(nc, in_maps, list(range(NCORES)))
    return res


def kernel(content_feat, style_feat, content_seg, style_seg, num_labels):
    L = int(num_labels)
    B, N, H, W = content_feat.shape
    M = H * W
    assert B == 1 and N == 256

    c = np.asarray(content_feat, dtype=np.float32).reshape(N, M)
    s = np.asarray(style_feat, dtype=np.float32).reshape(N, M)
    seg_c = np.asarray(content_seg).reshape(M).astype(np.int64)
    seg_s = np.asarray(style_seg).reshape(M).astype(np.int64)

    order_c = np.argsort(seg_c, kind="stable")
    order_s = np.argsort(seg_s, kind="stable")
    counts_c = np.bincount(seg_c, minlength=L)[:L]
    counts_s = np.bincount(seg_s, minlength=L)[:L]

    def split_counts(cnt):
        base = cnt // NCORES
        out = np.tile(base[:, None], (1, NCORES))
        for l in range(L):
            out[l, : cnt[l] % NCORES] += 1
        return out

    cc = split_counts(counts_c)  # (L, NCORES)
    cs = split_counts(counts_s)

    C = _round_up(max(cc.max(), cs.max()), 128)
    P = L * C
    # per-label live tile counts (same across cores; SPMD program)
    T1c = [max(1, -(-int(cc[l].max()) // 128)) for l in range(L)]
    T1s = [max(1, -(-int(cs[l].max()) // 128)) for l in range(L)]

    cT8 = np.ascontiguousarray(c.T).astype(E3)  # (M, N)
    sT8 = np.ascontiguousarray(s.T).astype(E3)

    def build_gathers(xT, order, counts, core_counts):
        lab_pos = np.concatenate(([0], np.cumsum(counts)))
        arrs = [np.zeros((P, N + 1), dtype=E3) for _ in range(NCORES)]
        for l in range(L):
            off = lab_pos[l]
            for k in range(NCORES):
                m = int(core_counts[l, k])
                if m:
                    a = arrs[k]
                    a[l * C : l * C + m, :N] = xT[order[off : off + m]]
                    a[l * C : l * C + m, N] = 1.0
                off += m
        return arrs

    gc_arrs = build_gathers(cT8, order_c, counts_c, cc)
    gs_arrs = build_gathers(sT8, order_s, counts_s, cs)
    del sT8

    # kick off phase-2 build + a dummy warm-up run in the background so its
    # NEFF compile overlaps phase 1's (wall-clock only; device results of the
    # dummy run are discarded). Falls back to the serial path on any failure.
    p2_box = {}

    def _precompile_p2():
        try:
            nc2 = _build_phase2(L, C, N)
            if PRECOMPILE_WARM:
                # note: reads whatever is in the persistent SBUF region
                # (garbage); never writes it. Results are discarded.
                z = {"tq": np.zeros((128, L, 2, 2, 128), dtype=E4)}
                run_bass_kernel_spmd(nc2, [z] * NCORES, list(range(NCORES)))
            p2_box["nc"] = nc2
        except Exception as e:  # pragma: no cover - fallback path
            p2_box["err"] = e

    import threading

    p2_thread = threading.Thread(target=_precompile_p2, daemon=True)
    p2_thread.start()

    # swizzle for phase 1: per label, DMA groups of tiles, each group laid
    # out (128, KT, N+1) so DMA chunks are contiguous per SBUF partition;
    # only the per-label live tiles (T1x[l]) are materialized
    T1 = C // 128

    def swizzle(a, T1x):
        tiles = a.reshape(L, T1, 128, N + 1)
        out = np.empty((L, T1 * 128 * (N + 1)), dtype=a.dtype)
        for l in range(L):
            pos = 0
            t0 = 0
            for kt in _p1_groups(T1x[l]):
                n = kt * 128 * (N + 1)
                out[l, pos : pos + n] = tiles[l, t0 : t0 + kt].transpose(1, 0, 2).reshape(-1)
                pos += n
                t0 += kt
        return out

    # phase-2 moving operand, preloaded into persistent SBUF by phase 1
    g2_arrs = [
        np.ascontiguousarray(gc_arrs[k][:, :N].astype(E4).T) for k in range(NCORES)
    ]

    nc1p = _build_phase1(L, C, N, T1c, T1s)
    if TRACE:
        # keep the traced phase-1 profile free of the background warm-up run
        p2_thread.join()
    res1 = _run(
        nc1p,
        [
            {
                "gc": swizzle(gc_arrs[k], T1c),
                "gs": swizzle(gs_arrs[k], T1s),
                "g2": g2_arrs[k],
            }
            for k in range(NCORES)
        ],
        "p1",
    )

    # host: all-reduce moments, finish stats, cholesky, transforms (float64)
    PW = 2 * (N + 1) - 128
    sc_sum = np.zeros((L, 128, PW), dtype=np.float64)
    ss_sum = np.zeros((L, 128, PW), dtype=np.float64)
    for k in range(NCORES):
        sc_sum += res1.results[k]["sc"]
        ss_sum += res1.results[k]["ss"]

    def unpack(ssum, l):
        Sm = np.empty((N, N), dtype=np.float64)
        Sm[0:128, :] = ssum[l, :, 0:N]
        Sm[128:N, 128:N] = ssum[l, :, N + 1 : N + 129]
        Sm[128:N, 0:128] = Sm[0:128, 128:N].T
        sums = np.concatenate([ssum[l, :, N], ssum[l, :, PW - 1]], axis=0)
        return Sm, sums

    eyeN = np.eye(N, dtype=np.float64)
    T_all = np.zeros((L, N, N), dtype=np.float64)
    b_all = np.zeros((L, N), dtype=np.float64)
    valid = np.zeros(L, dtype=bool)

    try:
        from scipy.linalg import solve_triangular as _st

        def tri_inv(Lm):
            return _st(Lm, eyeN, lower=True)
    except ImportError:

        def tri_inv(Lm):
            return np.linalg.solve(Lm, eyeN)

    for l in range(L):
        ncnt = float(counts_c[l])
        nsnt = float(counts_s[l])
        v = (ncnt > 10) and (nsnt > 10) and (ncnt < 100.0 * nsnt) and (nsnt < 100.0 * ncnt)
        Tl, bl = eyeN, np.zeros(N)
        if v:
            Sc, sum_c = unpack(sc_sum, l)
            Ss, sum_s = unpack(ss_sum, l)
            mc = sum_c / max(ncnt, 1.0)
            ms = sum_s / max(nsnt, 1.0)
            cov_c = (Sc - ncnt * np.outer(mc, mc)) / max(max(ncnt, 1.0) - 1.0, 1.0)
            cov_s = (Ss - nsnt * np.outer(ms, ms)) / max(max(nsnt, 1.0) - 1.0, 1.0)
            try:
                Lc = np.linalg.cholesky(cov_c)
                Ls = np.linalg.cholesky(cov_s)
                Tl = Ls @ tri_inv(Lc)
                bl = ms - Tl @ mc
            except np.linalg.LinAlgError:
                v, Tl, bl = False, eyeN, np.zeros(N)
        T_all[l], b_all[l], valid[l] = Tl, bl, v

    # phase-2 inputs: delta-form transform T' = DELTA_SCALE*(T - I).
    # The bias b is added during host reconstruction. Invalid labels keep
    # T'=0 -> delta 0 (host also restores them exactly from the content).
    tq_np = np.zeros((128, L, 2, 2, 128), dtype=E4)
    for l in range(L):
        if not valid[l]:
            continue
        Tp = (DELTA_SCALE * (T_all[l] - eyeN)).astype(np.float32)
        for j in range(2):
            for i in range(2):
                tq_np[:, l, j, i, :] = Tp[
                    i * 128 : (i + 1) * 128, j * 128 : (j + 1) * 128
                ].T

    p2_thread.join()
    nc2p = p2_box.get("nc")
    if nc2p is None:
        nc2p = _build_phase2(L, C, N)
    res2 = _run(
        nc2p,
        [{"tq": tq_np} for k in range(NCORES)],
        "p2",
    )

    # assemble: out = content + delta/8 + b, gathered order -> sorted order
    # -> original pixel order
    cT32 = np.ascontiguousarray(c.T)
    inv_scale = np.float32(1.0 / DELTA_SCALE)
    b32 = b_all.astype(np.float32)
    sorted_pm = np.empty((M, N), dtype=np.float32)
    pos = 0
    for l in range(L):
        for k in range(NCORES):
            m = int(cc[l, k])
            if m:
                if valid[l]:
                    delta = np.asarray(
                        res2.results[k]["oc"].T[l * C : l * C + m], dtype=np.float32
                    )
                    sorted_pm[pos : pos + m] = (
                        cT32[order_c[pos : pos + m]] + delta * inv_scale + b32[l]
                    )
                else:
                    sorted_pm[pos : pos + m] = cT32[order_c[pos : pos + m]]
            pos += m

    # pixels whose label is outside [0, L) are untouched by the reference
    if pos < M:
        sorted_pm[pos:] = cT32[order_c[pos:]]

    final_pm = np.empty((M, N), dtype=np.float32)
    final_pm[order_c] = sorted_pm
    return np.ascontiguousarray(final_pm.T).reshape(B, N, H, W)


# revision 27
# speedup vs baseline: 1.1965x; 1.1965x over previous
"""CWCT (class-wise whitening/coloring transform) for Trainium2, 8 NeuronCores.

Strategy
--------
Pixels are counting-sorted by segment label on the host (pure data
movement); each label's pixel range is split contiguously across the 8
cores, zero-padded to a fixed per-(core,label) capacity C.

Device phase 1 (per core): for every label, accumulate the raw second
moment S_l = sum_p x_p x_p^T and the channel sums over that core's pixel
shard, for content and style, as grouped 128-pixel-contraction matmuls
into PSUM (fp8e3 operands — 4 mantissa bits, f32 accumulate). A
ones-column appended to the gathered arrays yields the channel sums for
free in the same matmuls. Content streams on the SP(sync) DMA ring,
style on the ACT(scalar) ring so both HWDGE queues pull concurrently.
Partial moments are evicted as f16 (safe: partial sums are ~4e3 max and
the host all-reduce runs in f64).

Host middle: all-reduce the (tiny) per-core partial moments, form
covariances, Cholesky factors, inv_Lc via triangular solve (float64).
The transform is shipped in DELTA form: T' = 8*(T - I) quantized e4m3
and b' = 8*b, where T = Ls @ inv_Lc.

Device phase 2 (per core): delta = T' @ x + b' with x in e4m3 and
T' the DoubleRow-fp8 stationary (contraction 256 = 128 partitions x 2
k-tiles per single matmul), delta evicted as fp8e3. 1 byte/element on
both the input and output streams.

Host end: out = content_f32 + delta/8 scattered back to original pixel
order. Reconstructing against the exact f32 content means the x
quantization error only survives through (T - I) (spectral norm ~0.12),
so e4m3 x costs ~0.1% end-to-end error.
"""

import numpy as np
import ml_dtypes

import concourse.bacc as bacc
import concourse.mybir as mybir
import concourse.tile as tile
from concourse.bass_utils import run_bass_kernel_spmd

NCORES = 8
E3 = ml_dtypes.float8_e3m4
E4 = ml_dtypes.float8_e4m3
F16 = np.float16

# set by test harness to capture profiles
TRACE = False
TRACE_DIR = "/tmp/cwct_trace"
LAST_NS = {}
# overlap phase-2's NEFF compile (background thread + dummy run) with phase 1
PRECOMPILE_WARM = True
DELTA_SCALE = 8.0


def _round_up(x, m):
    return (int(x) + m - 1) // m * m


P1_GK = 16  # phase-1 DMA group size in 128-px tiles (526KB per DMA, 4KB rows)


def _pin_xpin(nc, L, C):
    """Reserve the cross-NEFF persistent SBUF region holding the phase-2
    moving operand x: [128(k), L, 2(j), C] e4m3 = channel j*128+k of pixel
    l*C+px. The label/pair axes keep every matmul AP stride <= C (the
    matmul ISA static-pattern step field is 16-bit). Fixed top-of-SBUF
    offset, identical in both phases (same formula, same L*C). Phase 1
    writes it with its spare DMA bandwidth; phase 2 only reads it. SBUF
    contents persist across NEFF executions on this stack (probed); tile
    pools bump-allocate from the bottom and stay far below."""
    size = 2 * L * C  # bytes per partition (e4m3)
    off = (229344 - size) // 4096 * 4096
    return nc.alloc_sbuf_tensor_at(
        "xpin", [128, L, 2, C], mybir.dt.float8e4, offset=off
    )


def _p1_groups(T1):
    """Phase-1 DMA group tile counts per (feature, label): uniform small
    groups so the first matmul starts ~1.3us after the first DMA and the
    pipeline stays fed."""
    kts = []
    rem = T1
    while rem > 0:
        kts.append(min(P1_GK, rem))
        rem -= P1_GK
    return kts


def _build_phase1(L, C, N, T1c, T1s):
    """Inputs gc/gs: (L, LBLK) fp8e3, host-swizzled pixel-major gathered
    tiles (+ones column); per label, _p1_groups(T1x[l]) DMA groups each
    laid out (128, KT, N+1) so one DMA pulls KT*(N+1) contiguous bytes per
    SBUF partition. T1c/T1s give the per-label live tile count (trailing
    all-zero pad tiles are neither DMA'd nor matmul'd).
    Outputs sc/ss: (L, 128, 386) f16 per label row block:
    [:, 0:256]   = S[0:128, 0:256] (upper row block, all columns)
    [:, 256]     = channel sums for channels 0..127
    [:, 257:385] = S[128:256, 128:256] (lower-right block)
    [:, 385]     = channel sums for channels 128..255
    (S[128:256, 0:128] is recovered on the host as S[0:128,128:256].T)"""
    assert N == 256
    T1 = C // 128
    W = 2 * (N + 1) - 128  # 386
    LBLK = T1 * 128 * (N + 1)
    P2 = L * C
    nc = bacc.Bacc("TRN2", target_bir_lowering=False, debug=False, num_devices=NCORES)
    gc = nc.dram_tensor("gc", [L, LBLK], mybir.dt.float8e3, kind="ExternalInput")
    gs = nc.dram_tensor("gs", [L, LBLK], mybir.dt.float8e3, kind="ExternalInput")
    g2 = nc.dram_tensor("g2", [N, P2], mybir.dt.float8e4, kind="ExternalInput")
    sc = nc.dram_tensor("sc", [L, 128, W], mybir.dt.float16, kind="ExternalOutput")
    ss = nc.dram_tensor("ss", [L, 128, W], mybir.dt.float16, kind="ExternalOutput")

    with tile.TileContext(nc) as tc:
        xpin = _pin_xpin(nc, L, C)
        with (
            tc.tile_pool(name="gin", bufs=12) as gin,
            tc.tile_pool(name="out", bufs=4) as outp,
            tc.tile_pool(name="ps", bufs=8, space="PSUM") as psum,
        ):
            # content on the SP(sync) HWDGE ring, style on the ACT(scalar)
            # ring; per-label interleave keeps both rings streaming and the
            # PE alternating between the two moment chains.
            for l in range(L):
                for g_dram, o_dram, ineng, T1l in (
                    (gc, sc, nc.sync, T1c[l]),
                    (gs, ss, nc.scalar, T1s[l]),
                ):
                    ps0 = psum.tile([128, N + 1], mybir.dt.float32, tag="ps")
                    ps1 = psum.tile([128, 129], mybir.dt.float32, tag="ps")
                    n = 0
                    off = 0
                    for KT in _p1_groups(T1l):
                        t = gin.tile([128, P1_GK, N + 1], mybir.dt.float8e3, tag="g")
                        src = g_dram[l, off : off + 128 * KT * (N + 1)].rearrange(
                            "(p t c) -> p t c", p=128, t=KT, c=N + 1
                        )
                        ineng.dma_start(t[:, 0:KT, :], src)
                        off += 128 * KT * (N + 1)
                        for k in range(KT):
                            nc.tensor.matmul(
                                ps0[:], t[:, k, 0:128], t[:, k, :],
                                start=(n == 0), stop=(n == T1l - 1),
                            )
                            nc.tensor.matmul(
                                ps1[:], t[:, k, 128:256], t[:, k, 128 : N + 1],
                                start=(n == 0), stop=(n == T1l - 1),
                            )
                            n += 1
                    ob = outp.tile([128, W], mybir.dt.float16, tag="o")
                    nc.vector.tensor_copy(ob[:, 0 : N + 1], ps0[:])
                    nc.vector.tensor_copy(ob[:, N + 1 : W], ps1[:])
                    ineng.dma_start(o_dram[l], ob[:])
                # preload one label-sized chunk of phase-2's x into the
                # persistent SBUF region on each ring, behind this label's
                # moment loads: uses the DMA slack of the PE-bound moment
                # loop and finishes before the PE tail
                nc.sync.dma_start(
                    xpin[:, l, 0, :], g2[0:128, l * C : (l + 1) * C]
                )
                nc.scalar.dma_start(
                    xpin[:, l, 1, :], g2[128:256, l * C : (l + 1) * C]
                )
    nc.compile()
    return nc


def _build_phase2(L, C, N):
    """tq: (128, L, 2, 2, 128) e4m3 with tq[k,l,j,i,m] = T'_l[i*128+m, j*128+k]
        where T' = DELTA_SCALE*(T_l - I); the (j=2) axis is the DoubleRow
        k-tile pair, so one matmul contracts all 256 channels.
    oc: (N, L*C) fp8e3 delta output (channel-major, gathered order).
    The moving operand x is READ FROM THE PERSISTENT SBUF REGION that
    phase 1 preloaded (no input DMA at all). The bias DELTA_SCALE*b is
    folded into the host reconstruction, so evictions are pure
    PSUM->fp8 copies."""
    assert N == 256
    P2 = L * C
    assert C % 128 == 0

    nc = bacc.Bacc("TRN2", target_bir_lowering=False, debug=False, num_devices=NCORES)
    tq = nc.dram_tensor("tq", [128, L, 2, 2, 128], mybir.dt.float8e4, kind="ExternalInput")
    oc = nc.dram_tensor("oc", [N, P2], mybir.dt.float8e3, kind="ExternalOutput")

    with tile.TileContext(nc) as tc:
        xpin = _pin_xpin(nc, L, C)
        with (
            tc.tile_pool(name="const", bufs=1) as constp,
            tc.tile_pool(name="out", bufs=4) as outp,
            tc.tile_pool(name="ps", bufs=8, space="PSUM") as psum,
        ):
            # stationary transforms on the sync ring, per-label slices so
            # the first matmul only waits for label 0's 64KB
            tqt = constp.tile([128, L, 2, 2, 128], mybir.dt.float8e4)
            for l in range(L):
                nc.sync.dma_start(tqt[:, l], tq[:, l])

            ocr2 = oc[:].rearrange("(i k) x -> k i x", i=2)
            # output groups of 4096 px (4KB DMA descriptor rows); PSUM-bank-
            # limited sub-blocks of <=512 px per matmul (one DoubleRow matmul
            # contracts all 256 channels). Evictions rotate 2:1 over DVE/ACT.
            GSZ = 4096
            groups = []
            off = 0
            while off < C:
                g = min(GSZ, C - off)
                subs = []
                so = 0
                while so < g:
                    s = min(512, g - so)
                    subs.append((so, s))
                    so += s
                groups.append((off, g, subs))
                off += g
            ev = 0
            for l in range(L):
                for off, G, subs in groups:
                    # both i-chunks evict into one tile -> a single output
                    # DMA per group
                    ob = outp.tile([128, 2, GSZ], mybir.dt.float8e3, tag="o")
                    for i in range(2):
                        for so, S in subs:
                            base = off + so
                            ps = psum.tile([128, 512], mybir.dt.float32, tag="ps")
                            nc.tensor.matmul(
                                ps[:, 0:S], tqt[:, l, :, i, :],
                                xpin[:, l, :, base : base + S], start=True, stop=True,
                                perf_mode=mybir.MatmulPerfMode.DoubleRow,
                            )
                            if ev % 2 == 0:
                                nc.vector.tensor_copy(
                                    ob[:, i, so : so + S], ps[:, 0:S]
                                )
                            else:
                                nc.scalar.activation(
                                    ob[:, i, so : so + S], ps[:, 0:S],
                                    mybir.ActivationFunctionType.Identity,
                                )
                            ev += 1
                    nc.sync.dma_start(
                        ocr2[:, :, l * C + off : l * C + off + G], ob[:, :, 0:G]
                    )
    nc.compile()
    return nc


def _run(nc, in_maps, label):
    if TRACE:
        import os
        import shutil

        tdir = f"{TRACE_DIR}/{label}"
        shutil.rmtree(tdir, ignore_errors=True)
        os.makedirs(tdir, exist_ok=True)
        res = run_bass_kernel_spmd(
            nc, in_maps, list(range(NCORES)), trace=True, tmpdir=tdir
        )
        LAST_NS[label] = res.exec_time_ns
    else:
        res = run_bass_kernel_spmd(nc, in_maps, list(range(NCORES)))
    return res


def kernel(content_feat, style_feat, content_seg, style_seg, num_labels):
    L = int(num_labels)
    B, N, H, W = content_feat.shape
    M = H * W
    assert B == 1 and N == 256

    c = np.asarray(content_feat, dtype=np.float32).reshape(N, M)
    s = np.asarray(style_feat, dtype=np.float32).reshape(N, M)
    seg_c = np.asarray(content_seg).reshape(M).astype(np.int64)
    seg_s = np.asarray(style_seg).reshape(M).astype(np.int64)

    order_c = np.argsort(seg_c, kind="stable")
    order_s = np.argsort(seg_s, kind="stable")
    counts_c = np.bincount(seg_c, minlength=L)[:L]
    counts_s = np.bincount(seg_s, minlength=L)[:L]

    def split_counts(cnt):
        base = cnt // NCORES
        out = np.tile(base[:, None], (1, NCORES))
        for l in range(L):
            out[l, : cnt[l] % NCORES] += 1
        return out

    cc = split_counts(counts_c)  # (L, NCORES)
    cs = split_counts(counts_s)

    C = _round_up(max(cc.max(), cs.max()), 128)
    P = L * C
    # per-label live tile counts (same across cores; SPMD program)
    T1c = [max(1, -(-int(cc[l].max()) // 128)) for l in range(L)]
    T1s = [max(1, -(-int(cs[l].max()) // 128)) for l in range(L)]

    cT8 = np.ascontiguousarray(c.T).astype(E3)  # (M, N)
    sT8 = np.ascontiguousarray(s.T).astype(E3)

    def build_gathers(xT, order, counts, core_counts):
        lab_pos = np.concatenate(([0], np.cumsum(counts)))
        arrs = [np.zeros((P, N + 1), dtype=E3) for _ in range(NCORES)]
        for l in range(L):
            off = lab_pos[l]
            for k in range(NCORES):
                m = int(core_counts[l, k])
                if m:
                    a = arrs[k]
                    a[l * C : l * C + m, :N] = xT[order[off : off + m]]
                    a[l * C : l * C + m, N] = 1.0
                off += m
        return arrs

    gc_arrs = build_gathers(cT8, order_c, counts_c, cc)
    gs_arrs = build_gathers(sT8, order_s, counts_s, cs)
    del sT8

    # kick off phase-2 build + a dummy warm-up run in the background so its
    # NEFF compile overlaps phase 1's (wall-clock only; device results of the
    # dummy run are discarded). Falls back to the serial path on any failure.
    p2_box = {}

    def _precompile_p2():
        try:
            nc2 = _build_phase2(L, C, N)
            if PRECOMPILE_WARM:
                # note: reads whatever is in the persistent SBUF region
                # (garbage); never writes it. Results are discarded.
                z = {"tq": np.zeros((128, L, 2, 2, 128), dtype=E4)}
                run_bass_kernel_spmd(nc2, [z] * NCORES, list(range(NCORES)))
            p2_box["nc"] = nc2
        except Exception as e:  # pragma: no cover - fallback path
            p2_box["err"] = e

    import threading

    p2_thread = threading.Thread(target=_precompile_p2, daemon=True)
    p2_thread.start()

    # swizzle for phase 1: per label, DMA groups of tiles, each group laid
    # out (128, KT, N+1) so DMA chunks are contiguous per SBUF partition;
    # only the per-label live tiles (T1x[l]) are materialized
    T1 = C // 128

    def swizzle(a, T1x):
        tiles = a.reshape(L, T1, 128, N + 1)
        out = np.empty((L, T1 * 128 * (N + 1)), dtype=a.dtype)
        for l in range(L):
            pos = 0
            t0 = 0
            for kt in _p1_groups(T1x[l]):
                n = kt * 128 * (N + 1)
                out[l, pos : pos + n] = tiles[l, t0 : t0 + kt].transpose(1, 0, 2).reshape(-1)
                pos += n
                t0 += kt
        return out

    # phase-2 moving operand, preloaded into persistent SBUF by phase 1
    g2_arrs = [
        np.ascontiguousarray(gc_arrs[k][:, :N].astype(E4).T) for k in range(NCORES)
    ]

    nc1p = _build_phase1(L, C, N, T1c, T1s)
    if TRACE:
        # keep the traced phase-1 profile free of the background warm-up run
        p2_thread.join()
    res1 = _run(
        nc1p,
        [
            {
                "gc": swizzle(gc_arrs[k], T1c),
                "gs": swizzle(gs_arrs[k], T1s),
                "g2": g2_arrs[k],
            }
            for k in range(NCORES)
        ],
        "p1",
    )

    # host: all-reduce moments, finish stats, cholesky, transforms (float64)
    PW = 2 * (N + 1) - 128
    sc_sum = np.zeros((L, 128, PW), dtype=np.float64)
    ss_sum = np.zeros((L, 128, PW), dtype=np.float64)
    for k in range(NCORES):
        sc_sum += res1.results[k]["sc"]
        ss_sum += res1.results[k]["ss"]

    def unpack(ssum, l):
        Sm = np.empty((N, N), dtype=np.float64)
        Sm[0:128, :] = ssum[l, :, 0:N]
        Sm[128:N, 128:N] = ssum[l, :, N + 1 : N + 129]
        Sm[128:N, 0:128] = Sm[0:128, 128:N].T
        sums = np.concatenate([ssum[l, :, N], ssum[l, :, PW - 1]], axis=0)
        return Sm, sums

    eyeN = np.eye(N, dtype=np.float64)
    T_all = np.zeros((L, N, N), dtype=np.float64)
    b_all = np.zeros((L, N), dtype=np.float64)
    valid = np.zeros(L, dtype=bool)

    try:
        from scipy.linalg import solve_triangular as _st

        def tri_inv(Lm):
            return _st(Lm, eyeN, lower=True)
    except ImportError:

        def tri_inv(Lm):
            return np.linalg.solve(Lm, eyeN)

    for l in range(L):
        ncnt = float(counts_c[l])
        nsnt = float(counts_s[l])
        v = (ncnt > 10) and (nsnt > 10) and (ncnt < 100.0 * nsnt) and (nsnt < 100.0 * ncnt)
        Tl, bl = eyeN, np.zeros(N)
        if v:
            Sc, sum_c = unpack(sc_sum, l)
            Ss, sum_s = unpack(ss_sum, l)
            mc = sum_c / max(ncnt, 1.0)
            ms = sum_s / max(nsnt, 1.0)
            cov_c = (Sc - ncnt * np.outer(mc, mc)) / max(max(ncnt, 1.0) - 1.0, 1.0)
            cov_s = (Ss - nsnt * np.outer(ms, ms)) / max(max(nsnt, 1.0) - 1.0, 1.0)
            try:
                Lc = np.linalg.cholesky(cov_c)
                Ls = np.linalg.cholesky(cov_s)
                Tl = Ls @ tri_inv(Lc)
                bl = ms - Tl @ mc
            except np.linalg.LinAlgError:
                v, Tl, bl = False, eyeN, np.zeros(N)
        T_all[l], b_all[l], valid[l] = Tl, bl, v

    # phase-2 inputs: delta-form transform T' = DELTA_SCALE*(T - I).
    # The bias b is added during host reconstruction. Invalid labels keep
    # T'=0 -> delta 0 (host also restores them exactly from the content).
    tq_np = np.zeros((128, L, 2, 2, 128), dtype=E4)
    for l in range(L):
        if not valid[l]:
            continue
        Tp = (DELTA_SCALE * (T_all[l] - eyeN)).astype(np.float32)
        for j in range(2):
            for i in range(2):
                tq_np[:, l, j, i, :] = Tp[
                    i * 128 : (i + 1) * 128, j * 128 : (j + 1) * 128
                ].T

    p2_thread.join()
    nc2p = p2_box.get("nc")
    if nc2p is None:
        nc2p = _build_phase2(L, C, N)
    res2 = _run(
        nc2p,
        [{"tq": tq_np} for k in range(NCORES)],
        "p2",
    )

    # assemble: out = content + delta/8 + b, gathered order -> sorted order
    # -> original pixel order
    cT32 = np.ascontiguousarray(c.T)
    inv_scale = np.float32(1.0 / DELTA_SCALE)
    b32 = b_all.astype(np.float32)
    sorted_pm = np.empty((M, N), dtype=np.float32)
    pos = 0
    for l in range(L):
        for k in range(NCORES):
            m = int(cc[l, k])
            if m:
                if valid[l]:
                    delta = np.asarray(
                        res2.results[k]["oc"].T[l * C : l * C + m], dtype=np.float32
                    )
                    sorted_pm[pos : pos + m] = (
                        cT32[order_c[pos : pos + m]] + delta * inv_scale + b32[l]
                    )
                else:
                    sorted_pm[pos : pos + m] = cT32[order_c[pos : pos + m]]
            pos += m

    # pixels whose label is outside [0, L) are untouched by the reference
    if pos < M:
        sorted_pm[pos:] = cT32[order_c[pos:]]

    final_pm = np.empty((M, N), dtype=np.float32)
    final_pm[order_c] = sorted_pm
    return np.ascontiguousarray(final_pm.T).reshape(B, N, H, W)
